# revision 13
# baseline (speedup 1.0000x reference)
"""MoE adapter kernel for 8 Trainium2 NeuronCores.

Math (faithful to the reference): every token routes to its top-2 of 8
experts (gate = 2-layer MLP on the concat embedding); the output is the
softmax-weighted sum of the two selected experts' MLP outputs.  The
reference computes ALL experts densely and combines with weights that are
exactly zero for unselected experts, so sparse top-2 computation is
mathematically identical (4x fewer FLOPs).

Strategy:
  - Host: gate + top-2 routing in float64 (selection margins are ~5e-5,
    fp noise ~1e-6, so selection matches the fp32 reference), group the
    16384 (token, expert) pairs by expert, pad each expert's run to
    512-token blocks (always <= 39 blocks), and hand 5 blocks to each of
    the 8 cores.  Perfectly balanced by construction.
  - Device (SPMD, same program on all 8 cores; per-core weights/tokens
    arrive as input data): per 512-token block, a 2-layer MLP
    [512,5120]x[5120,4096] -> relu -> x[4096,2048], fp16 operands with
    fp32 PSUM accumulation, weights stationary / activations moving.
  - Host: scatter-add  w * (y + b2)  into the [8192, 2048] output.
"""

import os
import numpy as np

B = 8192
IN_DIM = 5120
HID = 4096
OUT_DIM = 2048
E = 8
NCORES = 8
KT1 = IN_DIM // 128          # 40 k-tiles, layer 1
HT = HID // 128              # 32 hid tiles
KT2 = HID // 128             # 32 k-tiles, layer 2
OT = OUT_DIM // 128          # 16 out tiles

LAST_RESULT = None           # BassKernelResults of the most recent run


def _build_bass(BLK, S):
    import concourse.bass as bass
    import concourse.mybir as mybir
    import concourse.tile as tile
    from concourse import bacc
    from concourse.bass import ts

    f16 = mybir.dt.float16
    f32 = mybir.dt.float32

    nc = bacc.Bacc("TRN2", target_bir_lowering=False, debug=False,
                   num_devices=NCORES)

    xt_d, w1_d, w2_d, b1_d, yt_d = [], [], [], [], []
    for s in range(S):
        xt_d.append(nc.dram_tensor(f"xt_{s}", [128, KT1 * BLK], f16,
                                   kind="ExternalInput"))
        w1_d.append(nc.dram_tensor(f"w1_{s}", [HT, 128, KT1 * 128], f16,
                                   kind="ExternalInput"))
        w2_d.append(nc.dram_tensor(f"w2_{s}", [OT, 128, KT2 * 128], f16,
                                   kind="ExternalInput"))
        b1_d.append(nc.dram_tensor(f"b1_{s}", [128, HT], f32,
                                   kind="ExternalInput"))
        yt_d.append(nc.dram_tensor(f"yt_{s}", [OT, 128, BLK], f32,
                                   kind="ExternalOutput"))

    relu = mybir.ActivationFunctionType.Relu

    wbufs = 3 if BLK <= 460 else 2       # SBUF headroom shrinks with BLK
    with tile.TileContext(nc) as tc:
        with (
            tc.tile_pool(name="xt", bufs=2) as xt_pool,
            tc.tile_pool(name="w1", bufs=wbufs) as w1_pool,
            tc.tile_pool(name="w2", bufs=wbufs) as w2_pool,
            tc.tile_pool(name="h", bufs=2) as h_pool,
            tc.tile_pool(name="b", bufs=2) as b_pool,
            tc.tile_pool(name="y", bufs=4) as y_pool,
            tc.tile_pool(name="ps1", bufs=2, space="PSUM") as ps1_pool,
            tc.tile_pool(name="ps2", bufs=2, space="PSUM") as ps2_pool,
        ):
            for s in range(S):
                xt = xt_pool.tile([128, KT1 * BLK], f16, tag="xt")
                b1t = b_pool.tile([128, HT], f32, tag="b1")
                nc.sync.dma_start(out=b1t[:], in_=b1_d[s].ap())

                h_sb = h_pool.tile([128, HT * BLK], f16, tag="h")
                for h in range(HT):
                    w1t = w1_pool.tile([128, KT1 * 128], f16, tag="w1")
                    nc.sync.dma_start(out=w1t[:], in_=w1_d[s].ap()[h])
                    ps = ps1_pool.tile([128, BLK], f32, tag="ps1")
                    for k in range(KT1):
                        if h == 0 and k % 10 == 0:
                            # xt arrives in chunks so the first matmul only
                            # gates on 1/4 of the slot's activations
                            cols = slice(k * BLK, (k + 10) * BLK)
                            nc.sync.dma_start(out=xt[:, cols],
                                              in_=xt_d[s].ap()[:, cols])
                        nc.tensor.matmul(ps[:], w1t[:, ts(k, 128)],
                                         xt[:, ts(k, BLK)],
                                         start=(k == 0), stop=(k == KT1 - 1))
                    # hT[h] = relu(psum + b1), cast to fp16
                    nc.scalar.activation(h_sb[:, ts(h, BLK)], ps[:], relu,
                                         bias=b1t[:, h:h + 1])

                for o in range(OT):
                    w2t = w2_pool.tile([128, KT2 * 128], f16, tag="w2")
                    nc.sync.dma_start(out=w2t[:], in_=w2_d[s].ap()[o])
                    ps2 = ps2_pool.tile([128, BLK], f32, tag="ps2")
                    for k in range(KT2):
                        nc.tensor.matmul(ps2[:], w2t[:, ts(k, 128)],
                                         h_sb[:, ts(k, BLK)],
                                         start=(k == 0), stop=(k == KT2 - 1))
                    yt_sb = y_pool.tile([128, BLK], f32, tag="y")
                    nc.vector.tensor_copy(yt_sb[:], ps2[:])
                    nc.sync.dma_start(out=yt_d[s].ap()[o], in_=yt_sb[:])

    nc.compile()
    return nc


_NC = {}


def _get_nc(blk, s):
    if (blk, s) not in _NC:
        _NC[(blk, s)] = _build_bass(blk, s)
    return _NC[(blk, s)]


def _route(X, gW1, gb1, gW2, gb2):
    """Top-2 routing computed in float64 on the host."""
    g = np.maximum(X.astype(np.float64) @ gW1.astype(np.float64)
                   + gb1.astype(np.float64), 0.0)
    logits = g @ gW2.astype(np.float64) + gb2.astype(np.float64)   # [B, E]
    top2 = np.argpartition(-logits, 1, axis=1)[:, :2]              # [B, 2]
    l2 = np.take_along_axis(logits, top2, axis=1)
    ew = np.exp(l2 - l2.max(axis=1, keepdims=True))
    wts = ew / ew.sum(axis=1, keepdims=True)                       # [B, 2]
    return top2, wts.astype(np.float32)


def kernel(id_emb, llm_emb, W1, b1, W2, b2, gW1, gb1, gW2, gb2):
    global LAST_RESULT
    from concourse.bass_utils import run_bass_kernel_spmd

    X = np.concatenate([np.asarray(id_emb, np.float32),
                        np.asarray(llm_emb, np.float32)], axis=1)  # [B, IN]
    W1 = np.asarray(W1, np.float32); b1 = np.asarray(b1, np.float32)
    W2 = np.asarray(W2, np.float32); b2 = np.asarray(b2, np.float32)

    top2, wts = _route(X, np.asarray(gW1), np.asarray(gb1),
                       np.asarray(gW2), np.asarray(gb2))

    # ---- group (token, expert) pairs into blk-token blocks per expert ----
    per_e = []
    for e in range(E):
        mask = (top2 == e)                # [B, 2]
        ids = np.nonzero(mask.any(axis=1))[0]
        w_e = wts[mask]                   # row-major -> token order
        per_e.append((ids, w_e))
    counts = [len(ids) for ids, _ in per_e]

    # pick blk minimizing the critical path  ceil(nblocks/8) * blk
    best = None
    for cand in range(384, 513, 4):
        nb = sum(-(-c // cand) for c in counts if c)
        s_cand = max(1, -(-nb // NCORES))
        crit = s_cand * cand
        if s_cand <= 12 and (best is None or crit < best[0]):
            best = (crit, cand, s_cand)
    _, blk, S = best
    force = os.environ.get("KERNEL_FORCE_BLK")
    if force:
        blk = int(force)
        nb = sum(-(-c // blk) for c in counts if c)
        S = max(1, -(-nb // NCORES))

    blocks = []                           # (expert, ids, w)
    for e in range(E):
        ids, w_e = per_e[e]
        for i in range(0, len(ids), blk):
            blocks.append((e, ids[i:i + blk], w_e[i:i + blk]))
    assert len(blocks) <= NCORES * S

    # ---- per-expert device-layout weight packs (built once, fp16) ----
    used = sorted({e for e, _, _ in blocks})
    w1p, w2p, b1p = {}, {}, {}
    for e in used:
        w1p[e] = np.ascontiguousarray(
            W1[e].reshape(KT1, 128, HT, 128).transpose(2, 1, 0, 3)
        ).reshape(HT, 128, KT1 * 128).astype(np.float16)
        w2p[e] = np.ascontiguousarray(
            W2[e].reshape(KT2, 128, OT, 128).transpose(2, 1, 0, 3)
        ).reshape(OT, 128, KT2 * 128).astype(np.float16)
        b1p[e] = np.ascontiguousarray(b1[e].reshape(HT, 128).T)

    zero_xt = np.zeros((128, KT1 * blk), np.float16)
    e0 = used[0]

    # ---- per-core input maps ----
    in_maps = [dict() for _ in range(NCORES)]
    for bi, (e, ids, w) in enumerate(blocks):
        c, s = bi % NCORES, bi // NCORES
        n = len(ids)
        xb = np.zeros((blk, IN_DIM), np.float32)
        xb[:n] = X[ids]
        xt = np.ascontiguousarray(
            xb.T.reshape(KT1, 128, blk).transpose(1, 0, 2)
        ).reshape(128, KT1 * blk).astype(np.float16)
        m = in_maps[c]
        m[f"xt_{s}"] = xt
        m[f"w1_{s}"] = w1p[e]
        m[f"w2_{s}"] = w2p[e]
        m[f"b1_{s}"] = b1p[e]
    for c in range(NCORES):               # dummy slots
        m = in_maps[c]
        for s in range(S):
            if f"xt_{s}" not in m:
                m[f"xt_{s}"] = zero_xt
                m[f"w1_{s}"] = w1p[e0]
                m[f"w2_{s}"] = w2p[e0]
                m[f"b1_{s}"] = b1p[e0]

    # ---- run on the 8 cores ----
    nc = _get_nc(blk, S)
    trace = bool(int(os.environ.get("KERNEL_TRACE", "0")))
    res = run_bass_kernel_spmd(nc, in_maps, list(range(NCORES)), trace=trace)
    LAST_RESULT = res

    # ---- combine:  out[t] += w * (y + b2[e])  in expert order ----
    out = np.zeros((B, OUT_DIM), np.float32)
    for bi, (e, ids, w) in enumerate(blocks):
        c, s = bi % NCORES, bi // NCORES
        yt = np.asarray(res.results[c][f"yt_{s}"])        # [OT, 128, blk]
        y = yt.transpose(2, 0, 1).reshape(blk, OUT_DIM)[:len(ids)]
        out[ids] += w[:, None] * (y + b2[e][None, :])
    return out
